# revision 1
# baseline (speedup 1.0000x reference)
"""GroupedQueryAttention on 8 Trainium2 NeuronCores.

Sharding: core c = 4*b + g handles batch b (of 2) and KV group g (of 4),
i.e. 4 query heads (512 q-dims) + one 128-dim K/V head. o_proj is computed
as per-group partials (transposed layout) and summed with a ReduceScatter
across the 4 cores of each batch; each core ends up with a 512-row chunk
of the transposed output, which the host reassembles.

All matmuls run in fp16 (1 PE cycle/row) with fp32 PSUM accumulation.
Layouts are chosen so softmax is computed over the PSUM free dim never
needing a transpose of the big P matrix:
  - projections produce Q^T/K^T directly (lhsT=W tile, rhs=x^T tile)
  - scores are computed as S^T = (K^T).T @ Q^T
  - exp(S^T) = P^T feeds A@V as lhsT directly
  - V carries an extra ones-column so the softmax denominator falls out
    of the A@V matmul for free; normalization is applied to the small
    A@V output rather than to P.
"""

import math
import sys

import numpy as np

sys.path.insert(0, "/opt/trn_rl_repo")

B = 2
T = 2048
D = 2048
HEADS = 16
GROUPS = 4
HD = 128  # head dim
M = HEADS // GROUPS  # heads per group = 4
GQ = M * HD  # q dims per group = 512
SCALE = 1.0 / math.sqrt(HD)
N_CORES = 8
TCH = 512  # t chunk
NTCH = T // TCH  # 4
NSB = T // 128  # 16 s blocks
NKS = D // 128  # 16 contraction steps for projections

_COMPILED = {}


def _build():
    import concourse.bass as bass
    import concourse.mybir as mybir
    import concourse.tile as tile
    from concourse import bacc
    from concourse.masks import make_identity

    f16 = mybir.dt.float16
    f32 = mybir.dt.float32
    Exp = mybir.ActivationFunctionType.Exp
    Identity = mybir.ActivationFunctionType.Identity
    Copy = mybir.ActivationFunctionType.Copy

    nc = bacc.Bacc("TRN2", target_bir_lowering=False, num_devices=N_CORES)

    xT = nc.declare_dram_parameter("xT", [D, T], f16, isOutput=False)
    wq = nc.declare_dram_parameter("wq", [D, GQ], f16, isOutput=False)
    wk = nc.declare_dram_parameter("wk", [D, HD], f16, isOutput=False)
    wv = nc.declare_dram_parameter("wv", [D, HD], f16, isOutput=False)
    wo = nc.declare_dram_parameter("wo", [GQ, D], f16, isOutput=False)
    bqs_d = nc.declare_dram_parameter("bqs", [128, M], f32, isOutput=False)
    bks_d = nc.declare_dram_parameter("bks", [128, 1], f32, isOutput=False)
    bvs_d = nc.declare_dram_parameter("bvs", [128, 1], f32, isOutput=False)
    bo4_d = nc.declare_dram_parameter("bo4", [128, D // 128], f32, isOutput=False)
    outT = nc.declare_dram_parameter("outT", [D, TCH], f32, isOutput=True)

    groups = [[0, 1, 2, 3], [4, 5, 6, 7]]

    with tile.TileContext(nc) as tc:
        with (
            tc.tile_pool(name="const", bufs=1) as const,
            tc.tile_pool(name="work", bufs=2) as work,
            tc.tile_pool(name="psum", bufs=1, space="PSUM") as psum,
            tc.tile_pool(name="dram", bufs=1, space="DRAM") as dram,
        ):
            ident = const.tile([128, 128], f16)
            make_identity(nc, ident)
            bqs = const.tile([128, M], f32)
            bks = const.tile([128, 1], f32)
            bvs = const.tile([128, 1], f32)
            bo4 = const.tile([128, D // 128], f32)
            nc.sync.dma_start(bqs[:], bqs_d[:])
            nc.sync.dma_start(bks[:], bks_d[:])
            nc.sync.dma_start(bvs[:], bvs_d[:])
            nc.sync.dma_start(bo4[:], bo4_d[:])

            xt = const.tile([128, NKS, T], f16)
            wq_sb = const.tile([128, NKS, GQ], f16)
            wk_sb = const.tile([128, NKS, HD], f16)
            wv_sb = const.tile([128, NKS, HD], f16)
            wo_sb = const.tile([128, M, D], f16)
            for i in range(NKS):
                nc.sync.dma_start(xt[:, i, :], xT[i * 128 : (i + 1) * 128, :])
                nc.sync.dma_start(wq_sb[:, i, :], wq[i * 128 : (i + 1) * 128, :])
                nc.sync.dma_start(wk_sb[:, i, :], wk[i * 128 : (i + 1) * 128, :])
                nc.sync.dma_start(wv_sb[:, i, :], wv[i * 128 : (i + 1) * 128, :])
            for h in range(M):
                nc.sync.dma_start(wo_sb[:, h, :], wo[h * 128 : (h + 1) * 128, :])

            qt = const.tile([128, M, T], f16)
            kt = const.tile([128, T], f16)
            vt_sb = const.tile([128, T], f16)
            v_sb = const.tile([128, NSB, 132], f16)

            # ---- projections ----
            for h in range(M):
                for tc_i in range(NTCH):
                    acc = psum.tile([128, TCH], f32, tag="acc", bufs=3, name="acc")
                    for ks in range(NKS):
                        nc.tensor.matmul(
                            acc[:],
                            wq_sb[:, ks, h * 128 : (h + 1) * 128],
                            xt[:, ks, tc_i * TCH : (tc_i + 1) * TCH],
                            start=(ks == 0),
                            stop=(ks == NKS - 1),
                        )
                    nc.vector.tensor_scalar(
                        qt[:, h, tc_i * TCH : (tc_i + 1) * TCH],
                        acc[:],
                        SCALE,
                        bqs[:, h : h + 1],
                        op0=mybir.AluOpType.mult,
                        op1=mybir.AluOpType.add,
                    )
            for tc_i in range(NTCH):
                acc = psum.tile([128, TCH], f32, tag="acc", bufs=3, name="acc")
                for ks in range(NKS):
                    nc.tensor.matmul(
                        acc[:],
                        wk_sb[:, ks, :],
                        xt[:, ks, tc_i * TCH : (tc_i + 1) * TCH],
                        start=(ks == 0),
                        stop=(ks == NKS - 1),
                    )
                nc.vector.tensor_scalar_add(
                    kt[:, tc_i * TCH : (tc_i + 1) * TCH], acc[:], bks[:, 0:1]
                )
            for tc_i in range(NTCH):
                acc = psum.tile([128, TCH], f32, tag="acc", bufs=3, name="acc")
                for ks in range(NKS):
                    nc.tensor.matmul(
                        acc[:],
                        wv_sb[:, ks, :],
                        xt[:, ks, tc_i * TCH : (tc_i + 1) * TCH],
                        start=(ks == 0),
                        stop=(ks == NKS - 1),
                    )
                nc.vector.tensor_scalar_add(
                    vt_sb[:, tc_i * TCH : (tc_i + 1) * TCH], acc[:], bvs[:, 0:1]
                )
            # V natural [s, hd] + ones column for the denominator
            for s in range(NSB):
                tp = psum.tile([128, 128], f16, tag="tp", bufs=1, name="tp")
                nc.tensor.transpose(tp[:], vt_sb[:, s * 128 : (s + 1) * 128], ident[:])
                nc.vector.tensor_copy(v_sb[:, s, 0:128], tp[:])
            nc.vector.memset(v_sb[:, :, 128:129], 1.0)

            # ---- attention + o_proj, streamed per t-chunk ----
            partial = dram.tile([NTCH * D, TCH], f32, tag="ptl", name="partial")
            for tc_i in range(NTCH):
                at = work.tile([128, M, TCH], f16, tag="at", bufs=2, name="at")
                for h in range(M):
                    opks = [
                        psum.tile([128, 129], f32, tag="opk", bufs=4, name=f"opk{i}")
                        for i in range(4)
                    ]
                    for s in range(NSB):
                        sps = psum.tile([128, TCH], f32, tag="acc", bufs=3, name="sps")
                        nc.tensor.matmul(
                            sps[:],
                            kt[:, s * 128 : (s + 1) * 128],
                            qt[:, h, tc_i * TCH : (tc_i + 1) * TCH],
                            start=True,
                            stop=True,
                        )
                        p_sb = work.tile([128, TCH], f16, tag="p", bufs=3, name="p_sb")
                        nc.scalar.activation(p_sb[:], sps[:], Exp)
                        for tb in range(4):
                            nc.tensor.matmul(
                                opks[tb][:, 0:129],
                                p_sb[:, tb * 128 : (tb + 1) * 128],
                                v_sb[:, s, 0:129],
                                start=(s == 0),
                                stop=(s == NSB - 1),
                            )
                    for tb in range(4):
                        opk = opks[tb]
                        off = 0
                        rcp = work.tile([128, 1], f32, tag="rcp", bufs=2, name="rcp")
                        nc.vector.reciprocal(rcp[:], opk[:, off + 128 : off + 129])
                        o_sb = work.tile([128, 128], f16, tag="osb", bufs=2, name="osb")
                        nc.vector.tensor_scalar_mul(
                            o_sb[:], opk[:, off : off + 128], rcp[:]
                        )
                        tp = psum.tile([128, 128], f16, tag="tp", bufs=1, name="tp")
                        nc.tensor.transpose(tp[:], o_sb[:], ident[:])
                        nc.vector.tensor_copy(
                            at[:, h, tb * 128 : (tb + 1) * 128], tp[:]
                        )
                # o_proj partial (transposed): partial^T[c, t] for this t-chunk
                for cb in range(D // 128):
                    pp = psum.tile([128, TCH], f32, tag="acc", bufs=3, name="pp")
                    for h in range(M):
                        nc.tensor.matmul(
                            pp[:],
                            wo_sb[:, h, cb * 128 : (cb + 1) * 128],
                            at[:, h, :],
                            start=(h == 0),
                            stop=(h == M - 1),
                        )
                    po_sb = work.tile([128, TCH], f32, tag="po", bufs=3, name="po_sb")
                    nc.vector.tensor_scalar_add(po_sb[:], pp[:], bo4[:, cb : cb + 1])
                    nc.sync.dma_start(
                        partial[
                            tc_i * D + cb * 128 : tc_i * D + (cb + 1) * 128, :
                        ],
                        po_sb[:],
                    )
            rs = dram.tile([D, TCH], f32, tag="rs", name="rs")
            nc.gpsimd.collective_compute(
                "ReduceScatter",
                mybir.AluOpType.add,
                replica_groups=groups,
                ins=[partial[:]],
                outs=[rs[:]],
            )
            nc.sync.dma_start(outT[:], rs[:])

    nc.compile()
    return nc


def _get_nc():
    if "nc" not in _COMPILED:
        _COMPILED["nc"] = _build()
    return _COMPILED["nc"]


def kernel(x, Wq, bq, Wk, bk, Wv, bv, Wo, bo):
    from concourse.bass_utils import run_bass_kernel_spmd

    x = np.asarray(x, np.float32)
    Wq = np.asarray(Wq, np.float32)
    Wk = np.asarray(Wk, np.float32)
    Wv = np.asarray(Wv, np.float32)
    Wo = np.asarray(Wo, np.float32)
    bq = np.asarray(bq, np.float32)
    bk = np.asarray(bk, np.float32)
    bv = np.asarray(bv, np.float32)
    bo = np.asarray(bo, np.float32)

    nc = _get_nc()

    in_maps = []
    for c in range(N_CORES):
        b, g = c // 4, c % 4
        in_maps.append(
            {
                "xT": np.ascontiguousarray(x[b].T).astype(np.float16),
                "wq": np.ascontiguousarray(
                    Wq[:, g * GQ : (g + 1) * GQ]
                ).astype(np.float16),
                "wk": np.ascontiguousarray(
                    Wk[:, g * HD : (g + 1) * HD]
                ).astype(np.float16),
                "wv": np.ascontiguousarray(
                    Wv[:, g * HD : (g + 1) * HD]
                ).astype(np.float16),
                "wo": np.ascontiguousarray(
                    Wo[g * GQ : (g + 1) * GQ, :]
                ).astype(np.float16),
                "bqs": np.ascontiguousarray(
                    (bq[g * GQ : (g + 1) * GQ] * SCALE).reshape(M, 128).T
                ),
                "bks": np.ascontiguousarray(
                    bk[g * HD : (g + 1) * HD].reshape(1, 128).T
                ),
                "bvs": np.ascontiguousarray(
                    bv[g * HD : (g + 1) * HD].reshape(1, 128).T
                ),
                "bo4": np.ascontiguousarray((bo / 4.0).reshape(D // 128, 128).T),
            }
        )

    res = run_bass_kernel_spmd(nc, in_maps, list(range(N_CORES)))
    _COMPILED["last_res"] = res

    out = np.empty((B, T, D), np.float32)
    for b in range(B):
        for r in range(4):
            out[b, r * TCH : (r + 1) * TCH, :] = res.results[4 * b + r]["outT"].T
    return out



# revision 4
# speedup vs baseline: 1.4249x; 1.4249x over previous
"""GroupedQueryAttention on 8 Trainium2 NeuronCores.

Sharding: core c = 4*b + r handles batch b (of 2) and token chunk r (512
of 2048 tokens) for Q/attention/o_proj over ALL 16 heads. K/V projections
are sharded by KV group: core r computes group g=r's K/V for all T, then
one small (1 MB) AllGather of K^T / V across each batch's 4 cores makes
every core independent for the rest of the kernel -- no output collective
(the old version ReduceScattered a 16.8 MB fp32 o_proj partial at the end,
which serialized ~200 us).

Per-group token order is core-local (own chunk first): softmax + A@V are
permutation-invariant over keys as long as K and V share the order, so no
reordering is needed.

All matmuls run in fp16 (1 PE cycle/row) with fp32 PSUM accumulation.
Layouts avoid transposing the big P matrix:
  - projections produce Q^T/K^T/V^T directly (lhsT=W block, rhs=x^T block)
  - scores are computed as S^T = (K^T).T @ Q^T
  - exp(S^T) = P^T feeds A@V as lhsT directly
  - V carries an extra ones-column so the softmax denominator falls out
    of the A@V matmul for free
  - o_proj bias is added via an identity-matmul of a broadcast bias row
"""

import math
import sys

import numpy as np

sys.path.insert(0, "/opt/trn_rl_repo")

B = 2
T = 2048
D = 2048
HEADS = 16
GROUPS = 4
HD = 128  # head dim
M = HEADS // GROUPS  # heads per group = 4
SCALE = 1.0 / math.sqrt(HD)
N_CORES = 8
TCH = 512  # token chunk per core
NTCH = T // TCH  # 4
NSB = T // 128  # 16 key blocks
NKS = D // 128  # 16 contraction steps for projections
NNB = D // TCH  # 4 o_proj output column blocks

_COMPILED = {}


def _build():
    import concourse.bass as bass
    import concourse.mybir as mybir
    import concourse.tile as tile
    from concourse import bacc
    from concourse.masks import make_identity

    f16 = mybir.dt.float16
    f32 = mybir.dt.float32
    Exp = mybir.ActivationFunctionType.Exp

    nc = bacc.Bacc("TRN2", target_bir_lowering=False, num_devices=N_CORES)

    # x^T in 4 chunk-column blocks; slot 0 is always this core's own chunk
    xcb_d = nc.declare_dram_parameter("xcb", [NTCH * D, TCH], f16, isOutput=False)
    # weights pre-laid-out on host so every DMA is a full-row copy
    wq_d = nc.declare_dram_parameter("wq", [128, HEADS * NKS * 128], f16, isOutput=False)
    wk_d = nc.declare_dram_parameter("wk", [128, NKS * 128], f16, isOutput=False)
    wv_d = nc.declare_dram_parameter("wv", [128, NKS * 128], f16, isOutput=False)
    wo_d = nc.declare_dram_parameter("wo", [128, NNB * HEADS * TCH], f16, isOutput=False)
    bqs_d = nc.declare_dram_parameter("bqs", [128, HEADS], f32, isOutput=False)
    bks_d = nc.declare_dram_parameter("bks", [128, 1], f32, isOutput=False)
    bvs_d = nc.declare_dram_parameter("bvs", [128, 1], f32, isOutput=False)
    bob_d = nc.declare_dram_parameter("bob", [128, D], f16, isOutput=False)
    out_d = nc.declare_dram_parameter("out", [TCH, D], f32, isOutput=True)

    groups = [[0, 1, 2, 3], [4, 5, 6, 7]]

    with tile.TileContext(nc) as tc:
        with (
            tc.tile_pool(name="const", bufs=1) as const,
            tc.tile_pool(name="work", bufs=2) as work,
            tc.tile_pool(name="psum", bufs=1, space="PSUM") as psum,
            tc.tile_pool(name="dram", bufs=1, space="DRAM") as dram,
        ):
            ident = const.tile([128, 128], f16)
            make_identity(nc, ident)
            bqs = const.tile([128, HEADS], f32)
            bks = const.tile([128, 1], f32)
            bvs = const.tile([128, 1], f32)
            bob = const.tile([128, D], f16)
            nc.sync.dma_start(bqs[:], bqs_d[:])
            nc.sync.dma_start(bks[:], bks_d[:])
            nc.sync.dma_start(bvs[:], bvs_d[:])
            nc.sync.dma_start(bob[:], bob_d[:])

            wk_sb = const.tile([128, NKS, 128], f16)
            wv_sb = const.tile([128, NKS, 128], f16)
            nc.sync.dma_start(wk_sb[:], wk_d[:])
            nc.sync.dma_start(wv_sb[:], wv_d[:])

            # own-chunk x^T, kept resident (feeds K/V chunk 0 and all of Q)
            x_own = const.tile([128, NKS, TCH], f16)
            for ks in range(NKS):
                nc.sync.dma_start(x_own[:, ks, :], xcb_d[ks * 128 : (ks + 1) * 128, :])

            kt = const.tile([128, GROUPS, T], f16)  # gathered K^T
            v_sb = const.tile([128, GROUPS, NSB, 132], f16)  # gathered V + ones col
            qt = const.tile([128, HEADS, TCH], f16)  # own-chunk Q^T
            at = const.tile([128, HEADS, TCH], f16)  # own-chunk A^T

            kT_loc = dram.tile([128, T], f16, tag="kl", name="kT_loc")
            v_loc = dram.tile([T, 128], f16, tag="vl", name="v_loc")
            kT_g = dram.tile([GROUPS * 128, T], f16, tag="kg", name="kT_g")
            v_g = dram.tile([GROUPS * T, 128], f16, tag="vg", name="v_g")

            # ---- phase 1: K/V projection for own group, all T ----
            for tc2 in [1, 2, 3, 0]:
                kacc = psum.tile([128, TCH], f32, tag="acc", bufs=3, name="kacc")
                vacc = psum.tile([128, TCH], f32, tag="acc", bufs=3, name="vacc")
                for ks in range(NKS):
                    if tc2 == 0:
                        xb = x_own[:, ks, :]
                    else:
                        xbt = work.tile([128, TCH], f16, tag="xs", bufs=3, name="xbt")
                        nc.sync.dma_start(
                            xbt[:],
                            xcb_d[tc2 * D + ks * 128 : tc2 * D + (ks + 1) * 128, :],
                        )
                        xb = xbt[:]
                    nc.tensor.matmul(
                        kacc[:], wk_sb[:, ks, :], xb,
                        start=(ks == 0), stop=(ks == NKS - 1),
                    )
                    nc.tensor.matmul(
                        vacc[:], wv_sb[:, ks, :], xb,
                        start=(ks == 0), stop=(ks == NKS - 1),
                    )
                ktc = work.tile([128, TCH], f16, tag="ktc", bufs=2, name="ktc")
                nc.vector.tensor_scalar_add(ktc[:], kacc[:], bks[:, 0:1])
                nc.sync.dma_start(kT_loc[:, tc2 * TCH : (tc2 + 1) * TCH], ktc[:])
                vtc = work.tile([128, TCH], f16, tag="vtc", bufs=2, name="vtc")
                nc.vector.tensor_scalar_add(vtc[:], vacc[:], bvs[:, 0:1])
                for sb in range(4):
                    tp = psum.tile([128, 128], f16, tag="tp", bufs=1, name="tp")
                    nc.tensor.transpose(tp[:], vtc[:, sb * 128 : (sb + 1) * 128], ident[:])
                    vn = work.tile([128, 128], f16, tag="vn", bufs=2, name="vn")
                    nc.vector.tensor_copy(vn[:], tp[:])
                    nc.sync.dma_start(
                        v_loc[tc2 * TCH + sb * 128 : tc2 * TCH + (sb + 1) * 128, :],
                        vn[:],
                    )

            nc.gpsimd.collective_compute(
                "AllGather", mybir.AluOpType.bypass, replica_groups=groups,
                ins=[kT_loc[:]], outs=[kT_g[:]],
            )
            nc.gpsimd.collective_compute(
                "AllGather", mybir.AluOpType.bypass, replica_groups=groups,
                ins=[v_loc[:]], outs=[v_g[:]],
            )

            # ---- phase 2: Q projection (own chunk, all heads); overlaps AG ----
            for h in range(HEADS):
                wqh = work.tile([128, NKS * 128], f16, tag="wq", bufs=2, name="wqh")
                nc.sync.dma_start(wqh[:], wq_d[:, h * NKS * 128 : (h + 1) * NKS * 128])
                qacc = psum.tile([128, TCH], f32, tag="acc", bufs=3, name="qacc")
                for ks in range(NKS):
                    nc.tensor.matmul(
                        qacc[:], wqh[:, ks * 128 : (ks + 1) * 128], x_own[:, ks, :],
                        start=(ks == 0), stop=(ks == NKS - 1),
                    )
                nc.vector.tensor_scalar(
                    qt[:, h, :], qacc[:], SCALE, bqs[:, h : h + 1],
                    op0=mybir.AluOpType.mult, op1=mybir.AluOpType.add,
                )

            # unpack gathered K^T / V into SBUF
            for g in range(GROUPS):
                nc.sync.dma_start(kt[:, g, :], kT_g[g * 128 : (g + 1) * 128, :])
                for sb in range(NSB):
                    nc.sync.dma_start(
                        v_sb[:, g, sb, 0:128],
                        v_g[g * T + sb * 128 : g * T + (sb + 1) * 128, :],
                    )
            nc.vector.memset(v_sb[:, :, :, 128:129], 1.0)

            # ---- phase 3: attention for own chunk, all heads ----
            for g in range(GROUPS):
                for hh in range(M):
                    h = g * M + hh
                    opks = [
                        psum.tile([128, 129], f32, tag="opk", bufs=4, name=f"opk{i}")
                        for i in range(4)
                    ]
                    for s in range(NSB):
                        sps = psum.tile([128, TCH], f32, tag="acc", bufs=3, name="sps")
                        nc.tensor.matmul(
                            sps[:], kt[:, g, s * 128 : (s + 1) * 128], qt[:, h, :],
                            start=True, stop=True,
                        )
                        p_sb = work.tile([128, TCH], f16, tag="p", bufs=3, name="p_sb")
                        nc.scalar.activation(p_sb[:], sps[:], Exp)
                        for tb in range(4):
                            nc.tensor.matmul(
                                opks[tb][:, 0:129],
                                p_sb[:, tb * 128 : (tb + 1) * 128],
                                v_sb[:, g, s, 0:129],
                                start=(s == 0), stop=(s == NSB - 1),
                            )
                    for tb in range(4):
                        opk = opks[tb]
                        rcp = work.tile([128, 1], f32, tag="rcp", bufs=2, name="rcp")
                        nc.vector.reciprocal(rcp[:], opk[:, 128:129])
                        o_sb = work.tile([128, 128], f16, tag="osb", bufs=2, name="osb")
                        nc.vector.tensor_scalar_mul(o_sb[:], opk[:, 0:128], rcp[:])
                        tp = psum.tile([128, 128], f16, tag="tp", bufs=1, name="tp")
                        nc.tensor.transpose(tp[:], o_sb[:], ident[:])
                        nc.vector.tensor_copy(at[:, h, tb * 128 : (tb + 1) * 128], tp[:])

            # ---- phase 4: o_proj for own chunk, full D ----
            for nb in range(NNB):
                wob = work.tile([128, HEADS * TCH], f16, tag="wo", bufs=2, name="wob")
                nc.sync.dma_start(
                    wob[:], wo_d[:, nb * HEADS * TCH : (nb + 1) * HEADS * TCH]
                )
                for tb in range(4):
                    pp = psum.tile([128, TCH], f32, tag="acc", bufs=3, name="pp")
                    # bias row via identity matmul: out[m, n] += bob[m==*, n]
                    nc.tensor.matmul(
                        pp[:], ident[:], bob[:, nb * TCH : (nb + 1) * TCH],
                        start=True, stop=False,
                    )
                    for h in range(HEADS):
                        nc.tensor.matmul(
                            pp[:],
                            at[:, h, tb * 128 : (tb + 1) * 128],
                            wob[:, h * TCH : (h + 1) * TCH],
                            start=False, stop=(h == HEADS - 1),
                        )
                    ob = work.tile([128, TCH], f32, tag="ob", bufs=3, name="ob")
                    nc.vector.tensor_copy(ob[:], pp[:])
                    nc.sync.dma_start(
                        out_d[tb * 128 : (tb + 1) * 128, nb * TCH : (nb + 1) * TCH],
                        ob[:],
                    )

    nc.compile()
    return nc


def _get_nc():
    if "nc" not in _COMPILED:
        _COMPILED["nc"] = _build()
    return _COMPILED["nc"]


def kernel(x, Wq, bq, Wk, bk, Wv, bv, Wo, bo):
    from concourse.bass_utils import run_bass_kernel_spmd

    x = np.asarray(x, np.float32)
    Wq = np.asarray(Wq, np.float32)
    Wk = np.asarray(Wk, np.float32)
    Wv = np.asarray(Wv, np.float32)
    Wo = np.asarray(Wo, np.float32)
    bq = np.asarray(bq, np.float32)
    bk = np.asarray(bk, np.float32)
    bv = np.asarray(bv, np.float32)
    bo = np.asarray(bo, np.float32)

    nc = _get_nc()

    # shared across cores
    wq_h = np.ascontiguousarray(
        Wq.reshape(NKS, 128, HEADS, 128).transpose(1, 2, 0, 3).reshape(128, -1)
    ).astype(np.float16)
    wo_h = np.ascontiguousarray(
        Wo.reshape(HEADS, 128, NNB, TCH).transpose(1, 2, 0, 3).reshape(128, -1)
    ).astype(np.float16)
    bqs_h = np.ascontiguousarray((bq * SCALE).reshape(HEADS, 128).T)
    bob_h = np.ascontiguousarray(
        np.broadcast_to(bo.astype(np.float16), (128, D))
    )
    xT16 = [np.ascontiguousarray(x[b].T).astype(np.float16) for b in range(B)]

    wk_g, wv_g, bks_g, bvs_g = [], [], [], []
    for g in range(GROUPS):
        wk_g.append(
            np.ascontiguousarray(
                Wk[:, g * HD : (g + 1) * HD].reshape(NKS, 128, HD)
                .transpose(1, 0, 2).reshape(128, -1)
            ).astype(np.float16)
        )
        wv_g.append(
            np.ascontiguousarray(
                Wv[:, g * HD : (g + 1) * HD].reshape(NKS, 128, HD)
                .transpose(1, 0, 2).reshape(128, -1)
            ).astype(np.float16)
        )
        bks_g.append(np.ascontiguousarray(bk[g * HD : (g + 1) * HD].reshape(1, HD).T))
        bvs_g.append(np.ascontiguousarray(bv[g * HD : (g + 1) * HD].reshape(1, HD).T))

    in_maps = []
    for c in range(N_CORES):
        b, r = c // 4, c % 4
        order = [r] + [i for i in range(NTCH) if i != r]
        xcb = np.concatenate(
            [xT16[b][:, s * TCH : (s + 1) * TCH] for s in order], axis=0
        )
        in_maps.append(
            {
                "xcb": np.ascontiguousarray(xcb),
                "wq": wq_h,
                "wk": wk_g[r],
                "wv": wv_g[r],
                "wo": wo_h,
                "bqs": bqs_h,
                "bks": bks_g[r],
                "bvs": bvs_g[r],
                "bob": bob_h,
            }
        )

    res = run_bass_kernel_spmd(nc, in_maps, list(range(N_CORES)))
    _COMPILED["last_res"] = res

    out = np.empty((B, T, D), np.float32)
    for b in range(B):
        for r in range(NTCH):
            out[b, r * TCH : (r + 1) * TCH, :] = res.results[4 * b + r]["out"]
    return out


# revision 5
# speedup vs baseline: 1.5069x; 1.0576x over previous
"""GroupedQueryAttention on 8 Trainium2 NeuronCores.

Sharding: core c = 4*b + r handles batch b (of 2) and token chunk r (512
of 2048 tokens) for Q/attention/o_proj over ALL 16 heads. K/V projections
are sharded by KV group: core r computes group g=r's K/V for all T, then
one small (1 MB in / 4 MB out) AllGather across each batch's 4 cores makes
every core independent for the rest of the kernel -- no output collective.

K^T [128, T] and V-natural [T, 128] are packed into a single [256, 2048]
DRAM tile for the AllGather (V's rows are stored flat: DMA only checks
total size and both walk orders are linear).

Per-group token order is core-local (own chunk first): softmax + A@V are
permutation-invariant over keys as long as K and V share the order.

DMA is split across both HWDGE queues: the x^T stream (quad ks-blocks,
4 KB lines) on the Sync queue; weights, K/V writes, and unpacks on the
Activation queue, so the AllGather inputs never sit behind the stream.

All matmuls run in fp16 (1 PE cycle/row) with fp32 PSUM accumulation.
Layouts avoid transposing the big P matrix:
  - projections produce Q^T/K^T/V^T directly (lhsT=W block, rhs=x^T block)
  - scores are computed as S^T = (K^T).T @ Q^T
  - exp(S^T) = P^T feeds A@V as lhsT directly
  - V carries an extra ones-column so the softmax denominator falls out
    of the A@V matmul for free
  - o_proj bias is added via an identity-matmul of a broadcast bias row
"""

import math
import sys

import numpy as np

sys.path.insert(0, "/opt/trn_rl_repo")

B = 2
T = 2048
D = 2048
HEADS = 16
GROUPS = 4
HD = 128  # head dim
M = HEADS // GROUPS  # heads per group = 4
SCALE = 1.0 / math.sqrt(HD)
N_CORES = 8
TCH = 512  # token chunk per core
NTCH = T // TCH  # 4
NSB = T // 128  # 16 key blocks
NKS = D // 128  # 16 contraction steps for projections
NQ = NKS // 4  # 4 quad blocks for the x stream
NNB = D // TCH  # 4 o_proj output column blocks

_COMPILED = {}


def _build():
    import concourse.bass as bass
    import concourse.mybir as mybir
    import concourse.tile as tile
    from concourse import bacc
    from concourse.masks import make_identity

    f16 = mybir.dt.float16
    f32 = mybir.dt.float32
    Exp = mybir.ActivationFunctionType.Exp

    nc = bacc.Bacc("TRN2", target_bir_lowering=False, num_devices=N_CORES)

    # x^T as (slot, quad) row-blocks of [128, 2048]; slot 0 = own chunk
    xcb_d = nc.declare_dram_parameter("xcb", [NTCH * NQ * 128, 4 * TCH], f16,
                                      isOutput=False)
    wq_d = nc.declare_dram_parameter("wq", [128, HEADS * NKS * 128], f16,
                                     isOutput=False)
    wk_d = nc.declare_dram_parameter("wk", [128, NKS * 128], f16, isOutput=False)
    wv_d = nc.declare_dram_parameter("wv", [128, NKS * 128], f16, isOutput=False)
    wo_d = nc.declare_dram_parameter("wo", [128, NNB * HEADS * TCH], f16,
                                     isOutput=False)
    bqs_d = nc.declare_dram_parameter("bqs", [128, HEADS], f32, isOutput=False)
    bks_d = nc.declare_dram_parameter("bks", [128, 1], f32, isOutput=False)
    bvs_d = nc.declare_dram_parameter("bvs", [128, 1], f32, isOutput=False)
    bob_d = nc.declare_dram_parameter("bob", [128, D], f16, isOutput=False)
    out_d = nc.declare_dram_parameter("out", [TCH, D], f32, isOutput=True)

    groups = [[0, 1, 2, 3], [4, 5, 6, 7]]

    with tile.TileContext(nc) as tc:
        with (
            tc.tile_pool(name="const", bufs=1) as const,
            tc.tile_pool(name="work", bufs=2) as work,
            tc.tile_pool(name="psum", bufs=1, space="PSUM") as psum,
            tc.tile_pool(name="dram", bufs=1, space="DRAM") as dram,
        ):
            ident = const.tile([128, 128], f16)
            make_identity(nc, ident)
            bqs = const.tile([128, HEADS], f32)
            bks = const.tile([128, 1], f32)
            bvs = const.tile([128, 1], f32)
            bob = const.tile([128, D], f16)

            wk_sb = const.tile([128, NKS, 128], f16)
            wv_sb = const.tile([128, NKS, 128], f16)
            x_own = const.tile([128, NQ, 4 * TCH], f16)
            # weight/const loads on the Activation HWDGE queue
            nc.scalar.dma_start(wk_sb[:], wk_d[:])
            nc.scalar.dma_start(wv_sb[:], wv_d[:])
            nc.scalar.dma_start(bks[:], bks_d[:])
            nc.scalar.dma_start(bvs[:], bvs_d[:])
            nc.scalar.dma_start(bqs[:], bqs_d[:])
            for q in range(NQ):
                nc.scalar.dma_start(x_own[:, q, :], xcb_d[q * 128 : (q + 1) * 128, :])

            kt = const.tile([128, GROUPS, T], f16)  # gathered K^T
            v_sb = const.tile([128, GROUPS, NSB, 132], f16)  # gathered V + ones
            qt = const.tile([128, HEADS, TCH], f16)  # own-chunk Q^T
            at = const.tile([128, HEADS, TCH], f16)  # own-chunk A^T
            nc.vector.memset(v_sb[:, :, :, 128:129], 1.0)

            # merged AllGather payload: rows 0:128 K^T, rows 128:256 V flat
            kv_loc = dram.tile([256, T], f16, tag="kvl", name="kv_loc")
            kv_g = dram.tile([GROUPS * 256, T], f16, tag="kvg", name="kv_g")

            # ---- phase 1: K/V projection for own group, all T ----
            for tc2 in [1, 2, 3, 0]:
                kacc = psum.tile([128, TCH], f32, tag="acc", bufs=3, name="kacc")
                vacc = psum.tile([128, TCH], f32, tag="acc", bufs=3, name="vacc")
                for q in range(NQ):
                    if tc2 == 0:
                        x4 = x_own[:, q, :]
                    else:
                        x4t = work.tile([128, 4 * TCH], f16, tag="xs", bufs=3,
                                        name="x4t")
                        nc.sync.dma_start(
                            x4t[:],
                            xcb_d[(tc2 * NQ + q) * 128 : (tc2 * NQ + q + 1) * 128, :],
                        )
                        x4 = x4t[:]
                    for k2 in range(4):
                        ks = q * 4 + k2
                        xb = x4[:, k2 * TCH : (k2 + 1) * TCH]
                        nc.tensor.matmul(
                            kacc[:], wk_sb[:, ks, :], xb,
                            start=(ks == 0), stop=(ks == NKS - 1),
                        )
                        nc.tensor.matmul(
                            vacc[:], wv_sb[:, ks, :], xb,
                            start=(ks == 0), stop=(ks == NKS - 1),
                        )
                ktc = work.tile([128, TCH], f16, tag="ktc", bufs=2, name="ktc")
                nc.vector.tensor_scalar_add(ktc[:], kacc[:], bks[:, 0:1])
                nc.scalar.dma_start(kv_loc[0:128, tc2 * TCH : (tc2 + 1) * TCH], ktc[:])
                vtc = work.tile([128, TCH], f16, tag="vtc", bufs=2, name="vtc")
                nc.vector.tensor_scalar_add(vtc[:], vacc[:], bvs[:, 0:1])
                for sb in range(4):
                    tp = psum.tile([128, 128], f16, tag="tp", bufs=1, name="tp")
                    nc.tensor.transpose(tp[:], vtc[:, sb * 128 : (sb + 1) * 128],
                                        ident[:])
                    vn = work.tile([128, 128], f16, tag="vn", bufs=2, name="vn")
                    nc.vector.tensor_copy(vn[:], tp[:])
                    # V natural block -> flat rows of kv_loc (same linear order)
                    r0 = 128 + (tc2 * 4 + sb) * 8
                    nc.scalar.dma_start(kv_loc[r0 : r0 + 8, :], vn[:])

            nc.gpsimd.collective_compute(
                "AllGather", mybir.AluOpType.bypass, replica_groups=groups,
                ins=[kv_loc[:]], outs=[kv_g[:]],
            )

            # ---- phase 2: Q projection (own chunk, all heads); overlaps AG ----
            for h in range(HEADS):
                wqh = work.tile([128, NKS * 128], f16, tag="wq", bufs=2, name="wqh")
                nc.scalar.dma_start(wqh[:],
                                    wq_d[:, h * NKS * 128 : (h + 1) * NKS * 128])
                qacc = psum.tile([128, TCH], f32, tag="acc", bufs=3, name="qacc")
                for ks in range(NKS):
                    nc.tensor.matmul(
                        qacc[:], wqh[:, ks * 128 : (ks + 1) * 128],
                        x_own[:, ks // 4, (ks % 4) * TCH : (ks % 4 + 1) * TCH],
                        start=(ks == 0), stop=(ks == NKS - 1),
                    )
                nc.vector.tensor_scalar(
                    qt[:, h, :], qacc[:], SCALE, bqs[:, h : h + 1],
                    op0=mybir.AluOpType.mult, op1=mybir.AluOpType.add,
                )

            # unpack gathered K^T / V into SBUF (sync queue, idle by now)
            for g in range(GROUPS):
                nc.sync.dma_start(kt[:, g, :], kv_g[g * 256 : g * 256 + 128, :])
                for sb in range(NSB):
                    r0 = g * 256 + 128 + sb * 8
                    nc.sync.dma_start(v_sb[:, g, sb, 0:128], kv_g[r0 : r0 + 8, :])

            # ---- phase 3: attention for own chunk, all heads ----
            for g in range(GROUPS):
                for hh in range(M):
                    h = g * M + hh
                    opks = [
                        psum.tile([128, 129], f32, tag="opk", bufs=4, name=f"opk{i}")
                        for i in range(4)
                    ]
                    for s in range(NSB):
                        sps = psum.tile([128, TCH], f32, tag="acc", bufs=3, name="sps")
                        nc.tensor.matmul(
                            sps[:], kt[:, g, s * 128 : (s + 1) * 128], qt[:, h, :],
                            start=True, stop=True,
                        )
                        p_sb = work.tile([128, TCH], f16, tag="p", bufs=3, name="p_sb")
                        nc.scalar.activation(p_sb[:], sps[:], Exp)
                        for tb in range(4):
                            nc.tensor.matmul(
                                opks[tb][:, 0:129],
                                p_sb[:, tb * 128 : (tb + 1) * 128],
                                v_sb[:, g, s, 0:129],
                                start=(s == 0), stop=(s == NSB - 1),
                            )
                    for tb in range(4):
                        opk = opks[tb]
                        rcp = work.tile([128, 1], f32, tag="rcp", bufs=2, name="rcp")
                        nc.vector.reciprocal(rcp[:], opk[:, 128:129])
                        o_sb = work.tile([128, 128], f16, tag="osb", bufs=2, name="osb")
                        nc.vector.tensor_scalar_mul(o_sb[:], opk[:, 0:128], rcp[:])
                        tp = psum.tile([128, 128], f16, tag="tp", bufs=1, name="tp")
                        nc.tensor.transpose(tp[:], o_sb[:], ident[:])
                        nc.vector.tensor_copy(at[:, h, tb * 128 : (tb + 1) * 128],
                                              tp[:])

            # ---- phase 4: o_proj for own chunk, full D ----
            nc.scalar.dma_start(bob[:], bob_d[:])
            for nb in range(NNB):
                wob = work.tile([128, HEADS * TCH], f16, tag="wo", bufs=2, name="wob")
                nc.scalar.dma_start(
                    wob[:], wo_d[:, nb * HEADS * TCH : (nb + 1) * HEADS * TCH]
                )
                for tb in range(4):
                    pp = psum.tile([128, TCH], f32, tag="acc", bufs=3, name="pp")
                    # bias row via identity matmul: out[m, n] += bob[m, n]
                    nc.tensor.matmul(
                        pp[:], ident[:], bob[:, nb * TCH : (nb + 1) * TCH],
                        start=True, stop=False,
                    )
                    for h in range(HEADS):
                        nc.tensor.matmul(
                            pp[:],
                            at[:, h, tb * 128 : (tb + 1) * 128],
                            wob[:, h * TCH : (h + 1) * TCH],
                            start=False, stop=(h == HEADS - 1),
                        )
                    ob = work.tile([128, TCH], f32, tag="ob", bufs=3, name="ob")
                    nc.vector.tensor_copy(ob[:], pp[:])
                    nc.sync.dma_start(
                        out_d[tb * 128 : (tb + 1) * 128, nb * TCH : (nb + 1) * TCH],
                        ob[:],
                    )

    nc.compile()
    return nc


def _get_nc():
    if "nc" not in _COMPILED:
        _COMPILED["nc"] = _build()
    return _COMPILED["nc"]


def kernel(x, Wq, bq, Wk, bk, Wv, bv, Wo, bo):
    from concourse.bass_utils import run_bass_kernel_spmd

    x = np.asarray(x, np.float32)
    Wq = np.asarray(Wq, np.float32)
    Wk = np.asarray(Wk, np.float32)
    Wv = np.asarray(Wv, np.float32)
    Wo = np.asarray(Wo, np.float32)
    bq = np.asarray(bq, np.float32)
    bk = np.asarray(bk, np.float32)
    bv = np.asarray(bv, np.float32)
    bo = np.asarray(bo, np.float32)

    nc = _get_nc()

    # shared across cores
    wq_h = np.ascontiguousarray(
        Wq.reshape(NKS, 128, HEADS, 128).transpose(1, 2, 0, 3).reshape(128, -1)
    ).astype(np.float16)
    wo_h = np.ascontiguousarray(
        Wo.reshape(HEADS, 128, NNB, TCH).transpose(1, 2, 0, 3).reshape(128, -1)
    ).astype(np.float16)
    bqs_h = np.ascontiguousarray((bq * SCALE).reshape(HEADS, 128).T)
    bob_h = np.ascontiguousarray(np.broadcast_to(bo.astype(np.float16), (128, D)))
    # x^T per batch, pre-blocked into (chunk, quad) [128, 2048] row-blocks
    xq16 = []
    for b in range(B):
        xTb = x[b].T.astype(np.float16)  # [D, T]
        blocks = xTb.reshape(NKS, 128, NTCH, TCH).transpose(2, 0, 1, 3)
        # [chunk, ks, 128, TCH] -> quads: [chunk, quad, 128, 4*TCH]
        blocks = blocks.reshape(NTCH, NQ, 4, 128, TCH).transpose(0, 1, 3, 2, 4)
        xq16.append(np.ascontiguousarray(blocks.reshape(NTCH, NQ * 128, 4 * TCH)))

    wk_g, wv_g, bks_g, bvs_g = [], [], [], []
    for g in range(GROUPS):
        wk_g.append(
            np.ascontiguousarray(
                Wk[:, g * HD : (g + 1) * HD].reshape(NKS, 128, HD)
                .transpose(1, 0, 2).reshape(128, -1)
            ).astype(np.float16)
        )
        wv_g.append(
            np.ascontiguousarray(
                Wv[:, g * HD : (g + 1) * HD].reshape(NKS, 128, HD)
                .transpose(1, 0, 2).reshape(128, -1)
            ).astype(np.float16)
        )
        bks_g.append(np.ascontiguousarray(bk[g * HD : (g + 1) * HD].reshape(1, HD).T))
        bvs_g.append(np.ascontiguousarray(bv[g * HD : (g + 1) * HD].reshape(1, HD).T))

    in_maps = []
    for c in range(N_CORES):
        b, r = c // 4, c % 4
        order = [r] + [i for i in range(NTCH) if i != r]
        xcb = np.concatenate([xq16[b][s] for s in order], axis=0)
        in_maps.append(
            {
                "xcb": np.ascontiguousarray(xcb),
                "wq": wq_h,
                "wk": wk_g[r],
                "wv": wv_g[r],
                "wo": wo_h,
                "bqs": bqs_h,
                "bks": bks_g[r],
                "bvs": bvs_g[r],
                "bob": bob_h,
            }
        )

    res = run_bass_kernel_spmd(nc, in_maps, list(range(N_CORES)))
    _COMPILED["last_res"] = res

    out = np.empty((B, T, D), np.float32)
    for b in range(B):
        for r in range(NTCH):
            out[b, r * TCH : (r + 1) * TCH, :] = res.results[4 * b + r]["out"]
    return out


# revision 6
# speedup vs baseline: 1.5360x; 1.0193x over previous
"""GroupedQueryAttention on 8 Trainium2 NeuronCores.

Sharding: core c = 4*b + r handles batch b (of 2) and token chunk r (512
of 2048 tokens) for Q/attention/o_proj over ALL 16 heads. K/V projections
are sharded by KV group: core r computes group g=r's K/V for all T, then
one small (1 MB in / 4 MB out) AllGather across each batch's 4 cores makes
every core independent for the rest of the kernel -- no output collective.

AllGather payload kv_loc [256, 2048]: rows 0:128 K^T [hd, T]; rows
128:256 V in per-partition-major order (row 128+p, cols s*128+c holds
V[s*128+p, c]) so the consumer-side unpack is one fat [128, 2048] DMA
per group instead of 64 short-line DMAs.

Per-group token order is core-local (own chunk first): softmax + A@V are
permutation-invariant over keys as long as K and V share the order.

DMA queues: Sync carries the x^T stream, the wq stream, unpacks and
output writes; Activation carries x_own, wk/wv/biases, K/V AllGather
input writes, and the wo stream. This keeps the latency-critical
AllGather inputs and the Q-proj weight stream on independent queues.

PSUM: tag "big" = 2 bufs x [128, 1024] fp32 (scores for TWO key blocks
per buffer -> one batched exp instruction, halving Activation-engine
instruction+semaphore overhead, which paced v2's attention phase); tag
"opk" = 4 bufs x [128, 129] fp32 A@V accumulators, whose banks also host
the transient transpose outputs.

All matmuls run in fp16 (1 PE cycle/row) with fp32 PSUM accumulation.
Layouts avoid transposing the big P matrix:
  - projections produce Q^T/K^T/V^T directly (lhsT=W block, rhs=x^T block)
  - scores are computed as S^T = (K^T).T @ Q^T
  - exp(S^T) = P^T feeds A@V as lhsT directly
  - V carries an extra ones-column so the softmax denominator falls out
    of the A@V matmul for free
  - o_proj bias is added via an identity-matmul of a broadcast bias row
"""

import math
import sys

import numpy as np

sys.path.insert(0, "/opt/trn_rl_repo")

B = 2
T = 2048
D = 2048
HEADS = 16
GROUPS = 4
HD = 128  # head dim
M = HEADS // GROUPS  # heads per group = 4
SCALE = 1.0 / math.sqrt(HD)
N_CORES = 8
TCH = 512  # token chunk per core
NTCH = T // TCH  # 4
NSB = T // 128  # 16 key blocks
NKS = D // 128  # 16 contraction steps for projections
NQ = NKS // 4  # 4 quad blocks for the x stream
NNB = D // TCH  # 4 o_proj output column blocks

_COMPILED = {}


def _build():
    import concourse.bass as bass
    import concourse.mybir as mybir
    import concourse.tile as tile
    from concourse import bacc
    from concourse.masks import make_identity

    f16 = mybir.dt.float16
    f32 = mybir.dt.float32
    Exp = mybir.ActivationFunctionType.Exp

    nc = bacc.Bacc("TRN2", target_bir_lowering=False, num_devices=N_CORES)

    # x^T as (slot, quad) row-blocks of [128, 2048]; slot 0 = own chunk
    xcb_d = nc.declare_dram_parameter("xcb", [NTCH * NQ * 128, 4 * TCH], f16,
                                      isOutput=False)
    wq_d = nc.declare_dram_parameter("wq", [128, HEADS * NKS * 128], f16,
                                     isOutput=False)
    wk_d = nc.declare_dram_parameter("wk", [128, NKS * 128], f16, isOutput=False)
    wv_d = nc.declare_dram_parameter("wv", [128, NKS * 128], f16, isOutput=False)
    wo_d = nc.declare_dram_parameter("wo", [128, NNB * HEADS * TCH], f16,
                                     isOutput=False)
    bqs_d = nc.declare_dram_parameter("bqs", [128, HEADS], f32, isOutput=False)
    bks_d = nc.declare_dram_parameter("bks", [128, 1], f32, isOutput=False)
    bvs_d = nc.declare_dram_parameter("bvs", [128, 1], f32, isOutput=False)
    bob_d = nc.declare_dram_parameter("bob", [128, D], f16, isOutput=False)
    out_d = nc.declare_dram_parameter("out", [TCH, D], f32, isOutput=True)

    groups = [[0, 1, 2, 3], [4, 5, 6, 7]]

    with tile.TileContext(nc) as tc:
        with (
            tc.tile_pool(name="const", bufs=1) as const,
            tc.tile_pool(name="work", bufs=2) as work,
            tc.tile_pool(name="psum", bufs=1, space="PSUM") as psum,
            tc.tile_pool(name="dram", bufs=1, space="DRAM") as dram,
        ):
            ident = const.tile([128, 128], f16)
            make_identity(nc, ident)
            bqs = const.tile([128, HEADS], f32)
            bks = const.tile([128, 1], f32)
            bvs = const.tile([128, 1], f32)
            bob = const.tile([128, D], f16)

            wk_sb = const.tile([128, NKS, 128], f16)
            wv_sb = const.tile([128, NKS, 128], f16)
            x_own = const.tile([128, NQ, 4 * TCH], f16)
            # act-queue loads: needed by phase 1 / early phase 2
            nc.scalar.dma_start(wk_sb[:], wk_d[:])
            nc.scalar.dma_start(wv_sb[:], wv_d[:])
            nc.scalar.dma_start(bks[:], bks_d[:])
            nc.scalar.dma_start(bvs[:], bvs_d[:])
            nc.scalar.dma_start(bqs[:], bqs_d[:])
            for q in range(NQ):
                nc.scalar.dma_start(x_own[:, q, :], xcb_d[q * 128 : (q + 1) * 128, :])

            kt = const.tile([128, GROUPS, T], f16)  # gathered K^T
            v_sb = const.tile([128, GROUPS, NSB, 132], f16)  # gathered V + ones
            qt = const.tile([128, HEADS, TCH], f16)  # own-chunk Q^T
            at = const.tile([128, HEADS, TCH], f16)  # own-chunk A^T
            nc.vector.memset(v_sb[:, :, :, 128:129], 1.0)

            # AllGather payload: rows 0:128 K^T, rows 128:256 V p-major
            kv_loc = dram.tile([256, T], f16, tag="kvl", name="kv_loc")
            kv_g = dram.tile([GROUPS * 256, T], f16, tag="kvg", name="kv_g")

            # wq stream: sync queue, 4-deep, first two issued up front
            NWQB = 4
            wq_tiles = {}

            def issue_wq(h):
                wqh = work.tile([128, NKS * 128], f16, tag="wq", bufs=NWQB,
                                name="wqh", uniquify=True)
                nc.sync.dma_start(wqh[:],
                                  wq_d[:, h * NKS * 128 : (h + 1) * NKS * 128])
                wq_tiles[h] = wqh

            # ---- phase 1: K/V projection for own group, all T ----
            for tc2 in [1, 2, 3, 0]:
                kacc = psum.tile([128, TCH], f32, tag="big", bufs=2, name="kacc")
                vacc = psum.tile([128, TCH], f32, tag="big", bufs=2, name="vacc")
                for q in range(NQ):
                    if tc2 == 0:
                        x4 = x_own[:, q, :]
                    else:
                        x4t = work.tile([128, 4 * TCH], f16, tag="xs", bufs=3,
                                        name="x4t")
                        nc.sync.dma_start(
                            x4t[:],
                            xcb_d[(tc2 * NQ + q) * 128 : (tc2 * NQ + q + 1) * 128, :],
                        )
                        x4 = x4t[:]
                    for k2 in range(4):
                        ks = q * 4 + k2
                        xb = x4[:, k2 * TCH : (k2 + 1) * TCH]
                        nc.tensor.matmul(
                            kacc[:], wk_sb[:, ks, :], xb,
                            start=(ks == 0), stop=(ks == NKS - 1),
                        )
                        nc.tensor.matmul(
                            vacc[:], wv_sb[:, ks, :], xb,
                            start=(ks == 0), stop=(ks == NKS - 1),
                        )
                # prefetch two Q-proj weight tiles per chunk on the sync queue
                nwq = len(wq_tiles)
                if nwq < NWQB:
                    issue_wq(nwq)
                    issue_wq(nwq + 1)
                ktc = work.tile([128, TCH], f16, tag="ktc", bufs=2, name="ktc")
                nc.vector.tensor_scalar_add(ktc[:], kacc[:], bks[:, 0:1])
                nc.scalar.dma_start(kv_loc[0:128, tc2 * TCH : (tc2 + 1) * TCH], ktc[:])
                vtc = work.tile([128, TCH], f16, tag="vtc", bufs=2, name="vtc")
                nc.vector.tensor_scalar_add(vtc[:], vacc[:], bvs[:, 0:1])
                for sb in range(4):
                    sbg = tc2 * 4 + sb
                    tp = psum.tile([128, 128], f16, tag="opk", bufs=4, name="tp")
                    nc.tensor.transpose(tp[:], vtc[:, sb * 128 : (sb + 1) * 128],
                                        ident[:])
                    vn = work.tile([128, 128], f16, tag="vn", bufs=3, name="vn")
                    nc.vector.tensor_copy(vn[:], tp[:])
                    # V block, p-major: row 128+p, cols sbg*128..+128
                    nc.scalar.dma_start(
                        kv_loc[128:256, sbg * 128 : (sbg + 1) * 128], vn[:]
                    )

            nc.gpsimd.collective_compute(
                "AllGather", mybir.AluOpType.bypass, replica_groups=groups,
                ins=[kv_loc[:]], outs=[kv_g[:]],
            )

            # ---- phase 2: Q projection (own chunk, all heads); overlaps AG ----
            for h in range(HEADS):
                wqh = wq_tiles[h]
                qacc = psum.tile([128, TCH], f32, tag="big", bufs=2, name="qacc")
                for ks in range(NKS):
                    nc.tensor.matmul(
                        qacc[:], wqh[:, ks * 128 : (ks + 1) * 128],
                        x_own[:, ks // 4, (ks % 4) * TCH : (ks % 4 + 1) * TCH],
                        start=(ks == 0), stop=(ks == NKS - 1),
                    )
                nc.vector.tensor_scalar(
                    qt[:, h, :], qacc[:], SCALE, bqs[:, h : h + 1],
                    op0=mybir.AluOpType.mult, op1=mybir.AluOpType.add,
                )
                if h + NWQB < HEADS:
                    issue_wq(h + NWQB)

            # unpack gathered K^T / V into SBUF (sync queue)
            for g in range(GROUPS):
                nc.sync.dma_start(kt[:, g, :], kv_g[g * 256 : g * 256 + 128, :])
                nc.sync.dma_start(
                    v_sb[:, g, :, 0:128], kv_g[g * 256 + 128 : (g + 1) * 256, :]
                )

            # ---- phase 3: attention for own chunk, all heads ----
            for g in range(GROUPS):
                for hh in range(M):
                    h = g * M + hh
                    opks = [
                        psum.tile([128, 129], f32, tag="opk", bufs=4, name=f"opk{i}")
                        for i in range(4)
                    ]
                    for sp in range(NSB // 2):
                        sps2 = psum.tile([128, 2 * TCH], f32, tag="big", bufs=2,
                                         name="sps2")
                        for j in range(2):
                            s = sp * 2 + j
                            nc.tensor.matmul(
                                sps2[:, j * TCH : (j + 1) * TCH],
                                kt[:, g, s * 128 : (s + 1) * 128], qt[:, h, :],
                                start=True, stop=True,
                            )
                        p2 = work.tile([128, 2 * TCH], f16, tag="p", bufs=3,
                                       name="p2")
                        nc.scalar.activation(p2[:], sps2[:], Exp)
                        for j in range(2):
                            s = sp * 2 + j
                            for tb in range(4):
                                nc.tensor.matmul(
                                    opks[tb][:, 0:129],
                                    p2[:, j * TCH + tb * 128 : j * TCH + (tb + 1) * 128],
                                    v_sb[:, g, s, 0:129],
                                    start=(s == 0), stop=(s == NSB - 1),
                                )
                    for tb in range(4):
                        opk = opks[tb]
                        rcp = work.tile([128, 1], f32, tag="rcp", bufs=2, name="rcp")
                        nc.vector.reciprocal(rcp[:], opk[:, 128:129])
                        o_sb = work.tile([128, 128], f16, tag="osb", bufs=2, name="osb")
                        nc.vector.tensor_scalar_mul(o_sb[:], opk[:, 0:128], rcp[:])
                        tp = psum.tile([128, 128], f16, tag="opk", bufs=4, name="tpo")
                        nc.tensor.transpose(tp[:], o_sb[:], ident[:])
                        nc.vector.tensor_copy(at[:, h, tb * 128 : (tb + 1) * 128],
                                              tp[:])

            # ---- phase 4: o_proj for own chunk, full D ----
            nc.scalar.dma_start(bob[:], bob_d[:])
            for nb in range(NNB):
                wob = work.tile([128, HEADS * TCH], f16, tag="wo", bufs=2, name="wob")
                nc.scalar.dma_start(
                    wob[:], wo_d[:, nb * HEADS * TCH : (nb + 1) * HEADS * TCH]
                )
                for tb in range(4):
                    pp = psum.tile([128, TCH], f32, tag="big", bufs=2, name="pp")
                    # bias row via identity matmul: out[m, n] += bob[m, n]
                    nc.tensor.matmul(
                        pp[:], ident[:], bob[:, nb * TCH : (nb + 1) * TCH],
                        start=True, stop=False,
                    )
                    for h in range(HEADS):
                        nc.tensor.matmul(
                            pp[:],
                            at[:, h, tb * 128 : (tb + 1) * 128],
                            wob[:, h * TCH : (h + 1) * TCH],
                            start=False, stop=(h == HEADS - 1),
                        )
                    ob = work.tile([128, TCH], f32, tag="ob", bufs=3, name="ob")
                    nc.vector.tensor_copy(ob[:], pp[:])
                    nc.sync.dma_start(
                        out_d[tb * 128 : (tb + 1) * 128, nb * TCH : (nb + 1) * TCH],
                        ob[:],
                    )

    nc.compile()
    return nc


def _get_nc():
    if "nc" not in _COMPILED:
        _COMPILED["nc"] = _build()
    return _COMPILED["nc"]


def kernel(x, Wq, bq, Wk, bk, Wv, bv, Wo, bo):
    from concourse.bass_utils import run_bass_kernel_spmd

    x = np.asarray(x, np.float32)
    Wq = np.asarray(Wq, np.float32)
    Wk = np.asarray(Wk, np.float32)
    Wv = np.asarray(Wv, np.float32)
    Wo = np.asarray(Wo, np.float32)
    bq = np.asarray(bq, np.float32)
    bk = np.asarray(bk, np.float32)
    bv = np.asarray(bv, np.float32)
    bo = np.asarray(bo, np.float32)

    nc = _get_nc()

    # shared across cores
    wq_h = np.ascontiguousarray(
        Wq.reshape(NKS, 128, HEADS, 128).transpose(1, 2, 0, 3).reshape(128, -1)
    ).astype(np.float16)
    wo_h = np.ascontiguousarray(
        Wo.reshape(HEADS, 128, NNB, TCH).transpose(1, 2, 0, 3).reshape(128, -1)
    ).astype(np.float16)
    bqs_h = np.ascontiguousarray((bq * SCALE).reshape(HEADS, 128).T)
    bob_h = np.ascontiguousarray(np.broadcast_to(bo.astype(np.float16), (128, D)))
    # x^T per batch, pre-blocked into (chunk, quad) [128, 2048] row-blocks
    xq16 = []
    for b in range(B):
        xTb = x[b].T.astype(np.float16)  # [D, T]
        blocks = xTb.reshape(NKS, 128, NTCH, TCH).transpose(2, 0, 1, 3)
        # [chunk, ks, 128, TCH] -> quads: [chunk, quad, 128, 4*TCH]
        blocks = blocks.reshape(NTCH, NQ, 4, 128, TCH).transpose(0, 1, 3, 2, 4)
        xq16.append(np.ascontiguousarray(blocks.reshape(NTCH, NQ * 128, 4 * TCH)))

    wk_g, wv_g, bks_g, bvs_g = [], [], [], []
    for g in range(GROUPS):
        wk_g.append(
            np.ascontiguousarray(
                Wk[:, g * HD : (g + 1) * HD].reshape(NKS, 128, HD)
                .transpose(1, 0, 2).reshape(128, -1)
            ).astype(np.float16)
        )
        wv_g.append(
            np.ascontiguousarray(
                Wv[:, g * HD : (g + 1) * HD].reshape(NKS, 128, HD)
                .transpose(1, 0, 2).reshape(128, -1)
            ).astype(np.float16)
        )
        bks_g.append(np.ascontiguousarray(bk[g * HD : (g + 1) * HD].reshape(1, HD).T))
        bvs_g.append(np.ascontiguousarray(bv[g * HD : (g + 1) * HD].reshape(1, HD).T))

    in_maps = []
    for c in range(N_CORES):
        b, r = c // 4, c % 4
        order = [r] + [i for i in range(NTCH) if i != r]
        xcb = np.concatenate([xq16[b][s] for s in order], axis=0)
        in_maps.append(
            {
                "xcb": np.ascontiguousarray(xcb),
                "wq": wq_h,
                "wk": wk_g[r],
                "wv": wv_g[r],
                "wo": wo_h,
                "bqs": bqs_h,
                "bks": bks_g[r],
                "bvs": bvs_g[r],
                "bob": bob_h,
            }
        )

    res = run_bass_kernel_spmd(nc, in_maps, list(range(N_CORES)))
    _COMPILED["last_res"] = res

    out = np.empty((B, T, D), np.float32)
    for b in range(B):
        for r in range(NTCH):
            out[b, r * TCH : (r + 1) * TCH, :] = res.results[4 * b + r]["out"]
    return out
